# revision 5
# baseline (speedup 1.0000x reference)
"""Trainium2 Bass kernel for nn_RandProjector (histogram_binning).

Computes, for x [16384, 1024] and W [6400, 1024]:
    proj = x @ W.T                      # [S, D] -- never materialized in HBM
    per-column 20-bin histogram of proj (torch.histc semantics with
    mins/maxs as ranges), reshaped [100, 64, 20], L2-normalized over bins.

Strategy (8 NeuronCores, data-parallel over S):
  - Host folds the per-dim affine map into the problem: W rows are scaled
    by bins/width_d, so the matmul directly produces the "bin coordinate"
    u = (proj - min_d) * bins / width_d - bias_d, with bias added during
    the PSUM->SBUF staging pass.  x^T and W^T are pre-transposed on host
    (plain DMA loads, no xbar-transpose serialization).
  - Per 128-column tile of D: 32 fp16 matmuls accumulate into PSUM fp32;
    ScalarE stages PSUM->SBUF as *fp16* (Identity + per-partition bias).
  - cdf_b = #(u >= b) for integer edges b = 1..19 via fused
    compare+accumulate: NV edges on VectorE (tensor_scalar is_ge with an
    immediate scalar -- fp16 SBUF operands make it eligible for the DVE
    4x perf mode), NA edges on ScalarE (Sign activation, immediate bias).
  - Bin counts are cdf differences; bin 0 uses the constant shard total.
  - Histogram pieces AllReduce across the 8 cores as soon as their tiles
    finish (overlapping compute); the last piece is small so its
    collective is a short serial tail.  L2-normalize on device.
"""

import sys

if "/opt/trn_rl_repo" not in sys.path:
    sys.path.insert(0, "/opt/trn_rl_repo")

import numpy as np

S, IN_DIM = 16384, 1024
NUM_PROJ, PROJ_DIM, BINS = 100, 64, 20
D = NUM_PROJ * PROJ_DIM          # 6400
N_CORES = 8
S_SHARD = S // N_CORES           # 2048
NE = BINS - 1                    # 19 interior edges (b = 1..19)
NV = 15                          # edges handled by VectorE (b = 1..NV)
NA = NE - NV                     # edges handled by ScalarE (b = NV+1..19)

_CACHE = {}


def build(s_shard=S_SHARD, d=D, in_dim=IN_DIM, n_cores=N_CORES, debug=False):
    import concourse.bacc as bacc
    import concourse.bass as bass
    from concourse import mybir
    from concourse.tile import TileContext

    f32 = mybir.dt.float32
    f16 = mybir.dt.float16
    nt = d // 128                # 50
    kc_n = in_dim // 128         # 8
    chw = min(512, s_shard)      # matmul moving-operand width
    nch_n = s_shard // chw       # 4

    nc = bacc.Bacc("TRN2", target_bir_lowering=False, debug=False,
                   num_devices=n_cores)

    xT_d = nc.dram_tensor("xT16", [kc_n, 128, s_shard], f16,
                          kind="ExternalInput")
    wT_d = nc.dram_tensor("wT16", [kc_n, 128, d], f16, kind="ExternalInput")
    bias_d = nc.dram_tensor("bias", [128, nt], f32, kind="ExternalInput")
    out_d = nc.dram_tensor("out", [d, BINS], f32, kind="ExternalOutput")
    # pieces of the histogram all-reduce independently, overlapping compute;
    # the last piece is small so the trailing collective is short
    pieces = [(0, 22), (22, 42), (42, 48), (48, nt)]
    cc_ins, cc_outs = [], []
    for i, (t0, t1) in enumerate(pieces):
        cc_ins.append(nc.dram_tensor(f"cc_in{i}", [128, (t1 - t0) * BINS], f32))
        cc_outs.append(nc.dram_tensor(f"cc_out{i}", [128, (t1 - t0) * BINS],
                                      f32, addr_space="Shared"))
    if debug:
        dbg_hist = nc.dram_tensor("dbg_hist", [128, nt, BINS], f32,
                                  kind="ExternalOutput")
        dbg_cdf = nc.dram_tensor("dbg_cdf", [128, nt, BINS + 1], f32,
                                 kind="ExternalOutput")

    with TileContext(nc) as tc:
        with (
            tc.tile_pool(name="singles", bufs=1) as singles,
            tc.tile_pool(name="sp_pool", bufs=3) as sp_pool,
            tc.tile_pool(name="ps_p", bufs=2, space="PSUM") as ps_p,
        ):
            bias_sb = singles.tile([128, nt], f32)
            nc.sync.dma_start(out=bias_sb, in_=bias_d[:, :])

            # ScalarE Sign biases (-b for b = NV+1..19), one column each
            negb = singles.tile([128, NA], f32)
            for j in range(NA):
                nc.vector.memset(negb[:, j:j + 1], float(-(NV + 1 + j)))

            trash_v = singles.tile([128, s_shard], f16)
            trash_a = singles.tile([128, s_shard], f16)

            # per-engine cdf accumulators (separate tiles so Tile never
            # serializes VectorE against ScalarE on writes)
            acc_v = singles.tile([128, nt, NV], f32)
            acc_a = singles.tile([128, nt, NA], f32)

            # ---- Phase 0: plain DMA of pre-transposed x shard and W ----
            # x by nch chunk so the first matmuls start after ~1 MB; W in
            # d-chunks (first chunk small so tile 0 isn't gated on the
            # whole load).
            xT = singles.tile([128, kc_n, s_shard], f16)
            wT = singles.tile([128, kc_n, d], f16)
            for nch in range(nch_n):
                sl = slice(nch * chw, (nch + 1) * chw)
                for kc in range(kc_n):
                    nc.sync.dma_start(out=xT[:, kc, sl], in_=xT_d[kc, :, sl])
            d_bounds = [0]
            while d_bounds[-1] < d:
                nxt = 256 if d_bounds[-1] == 0 else 800
                d_bounds.append(min(d_bounds[-1] + nxt, d))
            for d0, d1 in zip(d_bounds[:-1], d_bounds[1:]):
                for kc in range(kc_n):
                    nc.sync.dma_start(out=wT[:, kc, d0:d1],
                                      in_=wT_d[kc, :, d0:d1])

            # normalization scratch (allocated up front, used per piece)
            cdfx = singles.tile([128, nt, BINS + 1], f32)
            nc.vector.memset(cdfx[:, :, 0:1], float(s_shard))
            nc.vector.memset(cdfx[:, :, BINS:BINS + 1], 0.0)
            hist = singles.tile([128, nt, BINS], f32)
            hsum = singles.tile([128, nt, BINS], f32)
            sq = singles.tile([128, nt, BINS], f32)
            n2 = singles.tile([128, nt], f32)
            y_t = singles.tile([128, nt], f32)
            iy = singles.tile([128, nt], f32)
            a_t = singles.tile([128, nt], f32)
            b_t = singles.tile([128, nt], f32)
            r_t = singles.tile([128, nt], f32)
            outn = singles.tile([128, nt, BINS], f32)
            out_v = out_d[:, :].rearrange("(t p) b -> p t b", p=128)

            def emit_cc(hi):
                """Combine cdf partials for tau in [t0, t1) and kick off the
                cross-core all-reduce (runs on DMA/CC queues in background)."""
                t0, t1 = pieces[hi]
                sl = slice(t0, t1)
                nc.vector.tensor_copy(cdfx[:, sl, 1:1 + NV], acc_v[:, sl])
                # ScalarE counts are sums of sign in {-1,0,1}:
                # cdf = 0.5*sgn + N/2
                nc.vector.tensor_scalar(
                    cdfx[:, sl, 1 + NV:BINS], acc_a[:, sl],
                    0.5, float(s_shard) / 2,
                    op0=mybir.AluOpType.mult, op1=mybir.AluOpType.add)
                nc.vector.tensor_tensor(
                    out=hist[:, sl], in0=cdfx[:, sl, 0:BINS],
                    in1=cdfx[:, sl, 1:BINS + 1],
                    op=mybir.AluOpType.subtract)
                if debug and hi == len(pieces) - 1:
                    nc.sync.dma_start(out=dbg_hist[:, :, :], in_=hist)
                    nc.sync.dma_start(out=dbg_cdf[:, :, :], in_=cdfx)
                nc.sync.dma_start(
                    out=cc_ins[hi][:, :],
                    in_=hist[:, sl].rearrange("p a b -> p (a b)"))
                nc.gpsimd.collective_compute(
                    "AllReduce",
                    mybir.AluOpType.add,
                    replica_groups=[list(range(n_cores))],
                    ins=[cc_ins[hi][:, :]],
                    outs=[cc_outs[hi][:, :]],
                )
                nc.sync.dma_start(
                    out=hsum[:, sl].rearrange("p a b -> p (a b)"),
                    in_=cc_outs[hi][:, :])

            hsum_g = singles.tile([128, nt, BINS], f32)

            def emit_norm(t0, t1, guard):
                """L2-normalize the summed histogram for tau in [t0, t1) and
                write the output slice. When `guard`, route hsum through a
                no-op add of last-tile accumulator data so the scheduler's
                cost model places the chain after the final tile -- the
                collective is then long finished and no engine FIFO stalls
                on it."""
                sl = slice(t0, t1)
                w = t1 - t0
                if guard:
                    g_ap = acc_a[:, nt - 1, NA - 1:NA]
                    g_b = bass.AP(tensor=g_ap.tensor, offset=g_ap.offset,
                                  ap=[g_ap.ap[0], [0, w], [0, BINS]])
                    nc.vector.scalar_tensor_tensor(
                        out=hsum_g[:, sl], in0=g_b, scalar=0.0,
                        in1=hsum[:, sl],
                        op0=mybir.AluOpType.mult, op1=mybir.AluOpType.add)
                    h_in = hsum_g
                else:
                    h_in = hsum
                nc.vector.tensor_tensor(out=sq[:, sl], in0=h_in[:, sl],
                                        in1=h_in[:, sl],
                                        op=mybir.AluOpType.mult)
                nc.vector.tensor_reduce(out=n2[:, sl], in_=sq[:, sl],
                                        axis=mybir.AxisListType.X,
                                        op=mybir.AluOpType.add)
                nc.scalar.sqrt(y_t[:, sl], n2[:, sl])
                nc.vector.reciprocal(iy[:, sl], y_t[:, sl])
                # one Newton step for rsqrt: r = iy * (1.5 - 0.5*n2*iy^2)
                nc.vector.tensor_tensor(out=a_t[:, sl], in0=iy[:, sl],
                                        in1=iy[:, sl],
                                        op=mybir.AluOpType.mult)
                nc.vector.tensor_tensor(out=b_t[:, sl], in0=a_t[:, sl],
                                        in1=n2[:, sl],
                                        op=mybir.AluOpType.mult)
                nc.vector.tensor_scalar(b_t[:, sl], b_t[:, sl], -0.5, 1.5,
                                        op0=mybir.AluOpType.mult,
                                        op1=mybir.AluOpType.add)
                nc.vector.tensor_tensor(out=r_t[:, sl], in0=iy[:, sl],
                                        in1=b_t[:, sl],
                                        op=mybir.AluOpType.mult)
                r_ap = r_t[:, sl]
                r_b = bass.AP(tensor=r_ap.tensor, offset=r_ap.offset,
                              ap=[r_ap.ap[0], r_ap.ap[1], [0, BINS]])
                nc.vector.tensor_tensor(out=outn[:, sl], in0=h_in[:, sl],
                                        in1=r_b, op=mybir.AluOpType.mult)
                nc.sync.dma_start(out=out_v[:, sl], in_=outn[:, sl])

            # ---- Phase 1: d-tiles ----
            for tau in range(nt):
                pp = ps_p.tile([128, s_shard], f32)
                for nch in range(nch_n):
                    for kc in range(kc_n):
                        nc.tensor.matmul(
                            pp[:, nch * chw:(nch + 1) * chw],
                            lhsT=wT[:, kc, tau * 128:(tau + 1) * 128],
                            rhs=xT[:, kc, nch * chw:(nch + 1) * chw],
                            start=(kc == 0),
                            stop=(kc == kc_n - 1),
                        )
                # Stage PSUM -> SBUF fp16 once, adding the per-dim bias:
                # u = proj * (bins/width) - min * (bins/width)  in [0, bins]
                sp = sp_pool.tile([128, s_shard], f16)
                nc.scalar.activation(
                    out=sp, in_=pp,
                    func=mybir.ActivationFunctionType.Identity,
                    bias=bias_sb[:, tau:tau + 1], scale=1.0)
                # ScalarE: edges b = NV+1 .. 19 via Sign(u - b)
                for j in range(NA):
                    nc.scalar.activation(
                        out=trash_a,
                        in_=sp,
                        func=mybir.ActivationFunctionType.Sign,
                        bias=negb[:, j:j + 1],
                        scale=1.0,
                        accum_out=acc_a[:, tau, j:j + 1],
                    )
                # VectorE: edges b = 1 .. NV via is_ge against an immediate
                for j in range(NV):
                    nc.vector.tensor_scalar(
                        trash_v,
                        sp,
                        float(1 + j),
                        None,
                        op0=mybir.AluOpType.is_ge,
                        op1=mybir.AluOpType.add,
                        accum_out=acc_v[:, tau, j:j + 1],
                    )
                for hi, (t0, t1) in enumerate(pieces):
                    if tau == t1 - 1:
                        emit_cc(hi)
            # all normalization at the end: earlier pieces' collectives have
            # long completed (guarded so the scheduler knows); only the last
            # piece's collective is actually waited on
            emit_norm(0, pieces[-2][1], guard=True)
            emit_norm(pieces[-1][0], pieces[-1][1], guard=False)

    nc.compile()
    return nc


def host_prep(x, W, mins, maxs, s_shard=S_SHARD, n_cores=N_CORES):
    d = W.shape[0]
    nt = d // 128
    kc_n = IN_DIM // 128
    mins64 = np.asarray(mins, dtype=np.float64)
    maxs64 = np.asarray(maxs, dtype=np.float64)
    scale = (float(BINS) / (maxs64 - mins64)).astype(np.float64)   # [d]
    # W rows scaled so matmul yields proj * bins/width directly
    Ws = np.asarray(W, dtype=np.float64) * scale[:, None]
    w16T = np.ascontiguousarray(Ws.T.astype(np.float16))           # [in, d]
    w16T = w16T.reshape(kc_n, 128, d)
    x16T = np.ascontiguousarray(np.asarray(x, dtype=np.float16).T)  # [in, S]
    x16T = x16T.reshape(kc_n, 128, S)
    bias = (-mins64 * scale).astype(np.float32)                    # [d]
    bias_l = np.ascontiguousarray(bias.reshape(nt, 128).T)         # [128, nt]
    in_maps = []
    for i in range(n_cores):
        in_maps.append({
            "xT16": np.ascontiguousarray(
                x16T[:, :, i * s_shard:(i + 1) * s_shard]),
            "wT16": w16T,
            "bias": bias_l,
        })
    return in_maps


def run(x, W, mins, maxs, trace=False, **trace_kw):
    """Returns (output [100, 64, 20] f32, BassKernelResults)."""
    from concourse.bass_utils import run_bass_kernel_spmd

    if "nc" not in _CACHE:
        _CACHE["nc"] = build()
    nc = _CACHE["nc"]
    in_maps = host_prep(x, W, mins, maxs)
    res = run_bass_kernel_spmd(nc, in_maps, core_ids=list(range(N_CORES)),
                               trace=trace, **trace_kw)
    out = res.results[0]["out"].reshape(NUM_PROJ, PROJ_DIM, BINS)
    return np.asarray(out, dtype=np.float32), res


def kernel(x, W, mins, maxs, num_of_projection=NUM_PROJ, bins=BINS):
    assert int(num_of_projection) == NUM_PROJ and int(bins) == BINS
    out, _ = run(x, W, mins, maxs, trace=False)
    return out


# revision 8
# speedup vs baseline: 1.6363x; 1.6363x over previous
"""Trainium2 Bass kernel for nn_RandProjector (histogram_binning).

Computes, for x [16384, 1024] and W [6400, 1024]:
    proj = x @ W.T                      # [S, D] -- never materialized in HBM
    per-column 20-bin histogram of proj (torch.histc semantics with
    mins/maxs as ranges), reshaped [100, 64, 20], L2-normalized over bins.

Strategy (8 NeuronCores, data-parallel over S):
  - Host pre-transposes x and W (plain multi-queue DMA loads, no xbar
    transpose serialization) and scales W rows by bins/width_d so the
    fp16 matmul produces proj' = proj * bins/width directly in PSUM fp32.
  - cdf_b = #(proj' >= e_b) for the 19 interior edges, via fused
    compare+accumulate passes reading PSUM directly.  The DVE accumulate
    path runs at 1x regardless of dtype (HW-measured), so VectorE uses a
    *custom dual-edge DVE op* (registered below):
        out = (x >= s0) + (x >= s1) * 4096 ; accum_out = sum(out)
    packing two exact counts (<= 2048 each) into one fp32 accumulator --
    12 edges in 6 passes.  Edges are paired (b, b+6) so the unpacked
    lo/hi counts land in contiguous cdf column ranges.  ScalarE handles
    the remaining 7 edges via Sign activation (+accum).
  - Bin counts are cdf differences; bin 0 uses the constant shard total.
  - Histogram pieces AllReduce across the 8 cores as soon as their tiles
    finish (overlapping compute); the last piece is small so the
    trailing collective is short.  L2-normalize on device.
"""

import sys

if "/opt/trn_rl_repo" not in sys.path:
    sys.path.insert(0, "/opt/trn_rl_repo")

import numpy as np

S, IN_DIM = 16384, 1024
NUM_PROJ, PROJ_DIM, BINS = 100, 64, 20
D = NUM_PROJ * PROJ_DIM          # 6400
N_CORES = 8
S_SHARD = S // N_CORES           # 2048
NE = BINS - 1                    # 19 interior edges (b = 1..19)
NP2 = 6                          # dual-edge passes on VectorE
NV = 2 * NP2                     # edges 1..12 on VectorE (pairs (b, b+6))
NA = NE - NV                     # edges 13..19 on ScalarE
PK = 4096.0                      # dual-edge packing factor

_CACHE = {}


def _cdf2_ref(in0, in1, s0, s1, imm2):
    x = in0.astype(np.float32)
    g = (x >= s0).astype(np.float32) + (x >= s1).astype(np.float32) * imm2
    return g, g.reshape(g.shape[0], -1).sum(axis=-1, keepdims=True)


def _register_cdf2():
    """Register the dual-edge compare+accumulate custom DVE op (idempotent)."""
    from operator import add

    from concourse import dve_ops
    from concourse.dve_spec import C0, C1, C2, Spec, Src0, Zero, lower
    from concourse.dve_uop import DveOpSpec

    if "CDF2_ANT" in dve_ops._SUB_OPCODE_FOR_NAME:
        return next(op for op in dve_ops.OPS if op.name == "CDF2_ANT")
    spec = Spec(
        body=(Src0 >= C0) + (Src0 >= C1) * C2,
        accum=add,
        accum_init=Zero,
        reference=_cdf2_ref,
    )
    row = max(dve_ops._SUB_OPCODE_FOR_NAME.values()) + 1
    assert row < 0x20
    sha = {}
    for ver in ("v3", "v4"):
        tmp = DveOpSpec(name="CDF2_ANT", opcode=row,
                        uops=lower(spec, ver=ver), rd1_en=False)
        sha[ver] = tmp.sha(ver)
    op = dve_ops.DveOp("CDF2_ANT", spec, subdim=False, uops_sha=sha)
    dve_ops.OPS.append(op)
    dve_ops.CUSTOM_DVE_SPECS[op.name] = op.spec
    dve_ops._SUB_OPCODE_FOR_NAME[op.name] = row
    return op


def build(s_shard=S_SHARD, d=D, in_dim=IN_DIM, n_cores=N_CORES, debug=False):
    import concourse.bacc as bacc
    import concourse.bass as bass
    from concourse import mybir
    from concourse.tile import TileContext

    cdf2 = _register_cdf2()

    f32 = mybir.dt.float32
    f16 = mybir.dt.float16
    i32 = mybir.dt.int32
    nt = d // 128                # 50
    kc_n = in_dim // 128         # 8
    chw = min(512, s_shard)      # matmul moving-operand width
    nch_n = s_shard // chw       # 4

    nc = bacc.Bacc("TRN2", target_bir_lowering=False, debug=False,
                   num_devices=n_cores)

    xT_d = nc.dram_tensor("xT16", [kc_n, 128, s_shard], f16,
                          kind="ExternalInput")
    wT_d = nc.dram_tensor("wT16", [kc_n, 128, d], f16, kind="ExternalInput")
    # VectorE dual-edge values (proj'-space): lo = b, hi = b+6, b = 1..6
    evlo_d = nc.dram_tensor("evlo", [128, nt, NP2], f32, kind="ExternalInput")
    evhi_d = nc.dram_tensor("evhi", [128, nt, NP2], f32, kind="ExternalInput")
    # ScalarE Sign biases: -e_b for b = 13..19
    nedges_d = nc.dram_tensor("nedges", [128, nt, NA], f32,
                              kind="ExternalInput")
    out_d = nc.dram_tensor("out", [d, BINS], f32, kind="ExternalOutput")
    # pieces of the histogram all-reduce independently, overlapping compute;
    # the last piece is small so the trailing collective is short
    pieces = [(0, 22), (22, 42), (42, 48), (48, nt)]
    cc_ins, cc_outs = [], []
    for i, (t0, t1) in enumerate(pieces):
        cc_ins.append(nc.dram_tensor(f"cc_in{i}", [128, (t1 - t0) * BINS], f32))
        cc_outs.append(nc.dram_tensor(f"cc_out{i}", [128, (t1 - t0) * BINS],
                                      f32, addr_space="Shared"))
    if debug:
        dbg_hist = nc.dram_tensor("dbg_hist", [128, nt, BINS], f32,
                                  kind="ExternalOutput")
        dbg_cdf = nc.dram_tensor("dbg_cdf", [128, nt, BINS + 1], f32,
                                 kind="ExternalOutput")

    with TileContext(nc) as tc:
        with (
            tc.tile_pool(name="singles", bufs=1) as singles,
            tc.tile_pool(name="ps_p", bufs=2, space="PSUM") as ps_p,
        ):
            evlo = singles.tile([128, nt, NP2], f32)
            evhi = singles.tile([128, nt, NP2], f32)
            nedges = singles.tile([128, nt, NA], f32)
            nc.sync.dma_start(out=evlo, in_=evlo_d[:, :, :])
            nc.sync.dma_start(out=evhi, in_=evhi_d[:, :, :])
            nc.sync.dma_start(out=nedges, in_=nedges_d[:, :, :])

            trash_v = singles.tile([128, s_shard], f32)
            trash_a = singles.tile([128, s_shard], f16)

            # per-engine cdf accumulators (separate tiles so Tile never
            # serializes VectorE against ScalarE on writes)
            accp = singles.tile([128, nt, NP2], f32)   # packed dual counts
            acc_a = singles.tile([128, nt, NA], f32)

            # ---- Phase 0: plain DMA of pre-transposed x shard and W ----
            xT = singles.tile([128, kc_n, s_shard], f16)
            wT = singles.tile([128, kc_n, d], f16)
            for nch in range(nch_n):
                sl = slice(nch * chw, (nch + 1) * chw)
                for kc in range(kc_n):
                    nc.sync.dma_start(out=xT[:, kc, sl], in_=xT_d[kc, :, sl])
            d_bounds = [0]
            while d_bounds[-1] < d:
                nxt = 256 if d_bounds[-1] == 0 else 800
                d_bounds.append(min(d_bounds[-1] + nxt, d))
            for d0, d1 in zip(d_bounds[:-1], d_bounds[1:]):
                for kc in range(kc_n):
                    nc.sync.dma_start(out=wT[:, kc, d0:d1],
                                      in_=wT_d[kc, :, d0:d1])

            # normalization / unpack scratch
            cdfx = singles.tile([128, nt, BINS + 1], f32)
            nc.vector.memset(cdfx[:, :, 0:1], float(s_shard))
            nc.vector.memset(cdfx[:, :, BINS:BINS + 1], 0.0)
            ip_t = singles.tile([128, nt, NP2], i32)
            ip2_t = singles.tile([128, nt, NP2], i32)
            hi_t = singles.tile([128, nt, NP2], f32)
            hist = singles.tile([128, nt, BINS], f32)
            hsum = singles.tile([128, nt, BINS], f32)
            sq = singles.tile([128, nt, BINS], f32)
            n2 = singles.tile([128, nt], f32)
            y_t = singles.tile([128, nt], f32)
            iy = singles.tile([128, nt], f32)
            a_t = singles.tile([128, nt], f32)
            b_t = singles.tile([128, nt], f32)
            r_t = singles.tile([128, nt], f32)
            outn = singles.tile([128, nt, BINS], f32)
            out_v = out_d[:, :].rearrange("(t p) b -> p t b", p=128)

            def emit_cc(hi):
                """Unpack cdf partials for tau in [t0, t1) and kick off the
                cross-core all-reduce (runs on DMA/CC queues in background)."""
                t0, t1 = pieces[hi]
                sl = slice(t0, t1)
                # unpack: lo = accp & 0xFFF (exact: accp is integer-valued
                # fp32 < 2^23, i32 convert exact), hi = (accp - lo) / 4096
                nc.vector.tensor_copy(ip_t[:, sl], accp[:, sl])
                nc.vector.tensor_scalar(ip2_t[:, sl], ip_t[:, sl], 0xFFF,
                                        None, op0=mybir.AluOpType.bitwise_and)
                nc.vector.tensor_copy(cdfx[:, sl, 1:1 + NP2], ip2_t[:, sl])
                nc.vector.tensor_tensor(out=hi_t[:, sl], in0=accp[:, sl],
                                        in1=cdfx[:, sl, 1:1 + NP2],
                                        op=mybir.AluOpType.subtract)
                nc.vector.tensor_scalar(cdfx[:, sl, 1 + NP2:1 + NV],
                                        hi_t[:, sl], 1.0 / PK, None,
                                        op0=mybir.AluOpType.mult)
                # ScalarE counts are sums of sign in {-1,0,1}:
                # cdf = 0.5*sgn + N/2
                nc.vector.tensor_scalar(
                    cdfx[:, sl, 1 + NV:BINS], acc_a[:, sl],
                    0.5, float(s_shard) / 2,
                    op0=mybir.AluOpType.mult, op1=mybir.AluOpType.add)
                nc.vector.tensor_tensor(
                    out=hist[:, sl], in0=cdfx[:, sl, 0:BINS],
                    in1=cdfx[:, sl, 1:BINS + 1],
                    op=mybir.AluOpType.subtract)
                if debug and hi == len(pieces) - 1:
                    nc.sync.dma_start(out=dbg_hist[:, :, :], in_=hist)
                    nc.sync.dma_start(out=dbg_cdf[:, :, :], in_=cdfx)
                nc.sync.dma_start(
                    out=cc_ins[hi][:, :],
                    in_=hist[:, sl].rearrange("p a b -> p (a b)"))
                nc.gpsimd.collective_compute(
                    "AllReduce",
                    mybir.AluOpType.add,
                    replica_groups=[list(range(n_cores))],
                    ins=[cc_ins[hi][:, :]],
                    outs=[cc_outs[hi][:, :]],
                )
                nc.sync.dma_start(
                    out=hsum[:, sl].rearrange("p a b -> p (a b)"),
                    in_=cc_outs[hi][:, :])

            hsum_g = singles.tile([128, nt, BINS], f32)

            def emit_norm(t0, t1, guard):
                """L2-normalize the summed histogram for tau in [t0, t1) and
                write the output slice. When `guard`, route hsum through a
                no-op add of last-tile accumulator data so the scheduler's
                cost model places the chain after the final tile -- the
                collective is then long finished and no engine FIFO stalls
                on it."""
                sl = slice(t0, t1)
                w = t1 - t0
                if guard:
                    g_ap = acc_a[:, nt - 1, NA - 1:NA]
                    g_b = bass.AP(tensor=g_ap.tensor, offset=g_ap.offset,
                                  ap=[g_ap.ap[0], [0, w], [0, BINS]])
                    nc.vector.scalar_tensor_tensor(
                        out=hsum_g[:, sl], in0=g_b, scalar=0.0,
                        in1=hsum[:, sl],
                        op0=mybir.AluOpType.mult, op1=mybir.AluOpType.add)
                    h_in = hsum_g
                else:
                    h_in = hsum
                nc.vector.tensor_tensor(out=sq[:, sl], in0=h_in[:, sl],
                                        in1=h_in[:, sl],
                                        op=mybir.AluOpType.mult)
                nc.vector.tensor_reduce(out=n2[:, sl], in_=sq[:, sl],
                                        axis=mybir.AxisListType.X,
                                        op=mybir.AluOpType.add)
                nc.scalar.sqrt(y_t[:, sl], n2[:, sl])
                nc.vector.reciprocal(iy[:, sl], y_t[:, sl])
                # one Newton step for rsqrt: r = iy * (1.5 - 0.5*n2*iy^2)
                nc.vector.tensor_tensor(out=a_t[:, sl], in0=iy[:, sl],
                                        in1=iy[:, sl],
                                        op=mybir.AluOpType.mult)
                nc.vector.tensor_tensor(out=b_t[:, sl], in0=a_t[:, sl],
                                        in1=n2[:, sl],
                                        op=mybir.AluOpType.mult)
                nc.vector.tensor_scalar(b_t[:, sl], b_t[:, sl], -0.5, 1.5,
                                        op0=mybir.AluOpType.mult,
                                        op1=mybir.AluOpType.add)
                nc.vector.tensor_tensor(out=r_t[:, sl], in0=iy[:, sl],
                                        in1=b_t[:, sl],
                                        op=mybir.AluOpType.mult)
                r_ap = r_t[:, sl]
                r_b = bass.AP(tensor=r_ap.tensor, offset=r_ap.offset,
                              ap=[r_ap.ap[0], r_ap.ap[1], [0, BINS]])
                nc.vector.tensor_tensor(out=outn[:, sl], in0=h_in[:, sl],
                                        in1=r_b, op=mybir.AluOpType.mult)
                nc.sync.dma_start(out=out_v[:, sl], in_=outn[:, sl])

            # ---- Phase 1: d-tiles ----
            for tau in range(nt):
                pp = ps_p.tile([128, s_shard], f32)
                for nch in range(nch_n):
                    for kc in range(kc_n):
                        nc.tensor.matmul(
                            pp[:, nch * chw:(nch + 1) * chw],
                            lhsT=wT[:, kc, tau * 128:(tau + 1) * 128],
                            rhs=xT[:, kc, nch * chw:(nch + 1) * chw],
                            start=(kc == 0),
                            stop=(kc == kc_n - 1),
                        )
                # ScalarE: edges b = NV+1 .. 19 via Sign(proj' - e_b), PSUM src
                for j in range(NA):
                    nc.scalar.activation(
                        out=trash_a,
                        in_=pp,
                        func=mybir.ActivationFunctionType.Sign,
                        bias=nedges[:, tau, j:j + 1],
                        scale=1.0,
                        accum_out=acc_a[:, tau, j:j + 1],
                    )
                # VectorE: edges b and b+6 per dual pass, PSUM src
                for j in range(NP2):
                    nc.vector._custom_dve(
                        cdf2,
                        out=trash_v,
                        in0=pp,
                        s0=evlo[:, tau, j:j + 1],
                        s1=evhi[:, tau, j:j + 1],
                        imm2=PK,
                        accum_out=accp[:, tau, j:j + 1],
                    )
                for hi, (t0, t1) in enumerate(pieces):
                    if tau == t1 - 1:
                        emit_cc(hi)
            # all normalization at the end: earlier pieces' collectives have
            # long completed (guarded so the scheduler knows); only the last
            # piece's collective is actually waited on
            emit_norm(0, pieces[-2][1], guard=True)
            emit_norm(pieces[-1][0], pieces[-1][1], guard=False)

    nc.compile()
    return nc


def host_prep(x, W, mins, maxs, s_shard=S_SHARD, n_cores=N_CORES):
    d = W.shape[0]
    nt = d // 128
    kc_n = IN_DIM // 128
    mins64 = np.asarray(mins, dtype=np.float64)
    maxs64 = np.asarray(maxs, dtype=np.float64)
    scale = float(BINS) / (maxs64 - mins64)                        # [d]
    # W rows scaled so the matmul yields proj' = proj * bins/width
    Ws = np.asarray(W, dtype=np.float64) * scale[:, None]
    w16T = np.ascontiguousarray(Ws.T.astype(np.float16))           # [in, d]
    w16T = w16T.reshape(kc_n, 128, d)
    x16T = np.ascontiguousarray(np.asarray(x, dtype=np.float16).T)  # [in, S]
    x16T = x16T.reshape(kc_n, 128, S)
    # edge b in proj'-space: e_b = b + min*scale
    base = mins64 * scale                                          # [d]
    evlo = np.empty((d, NP2), dtype=np.float32)
    evhi = np.empty((d, NP2), dtype=np.float32)
    nedges = np.empty((d, NA), dtype=np.float32)
    for j in range(NP2):
        evlo[:, j] = (1 + j) + base
        evhi[:, j] = (1 + j + NP2) + base
    for j in range(NA):
        nedges[:, j] = -((NV + 1 + j) + base)
    def lay(a):
        # [d, k] -> [128, nt, k]
        k = a.shape[1]
        return np.ascontiguousarray(a.reshape(nt, 128, k).transpose(1, 0, 2))
    evlo_l, evhi_l, nedges_l = lay(evlo), lay(evhi), lay(nedges)
    in_maps = []
    for i in range(n_cores):
        in_maps.append({
            "xT16": np.ascontiguousarray(
                x16T[:, :, i * s_shard:(i + 1) * s_shard]),
            "wT16": w16T,
            "evlo": evlo_l,
            "evhi": evhi_l,
            "nedges": nedges_l,
        })
    return in_maps


def run(x, W, mins, maxs, trace=False, **trace_kw):
    """Returns (output [100, 64, 20] f32, BassKernelResults)."""
    from concourse.bass_utils import run_bass_kernel_spmd

    if "nc" not in _CACHE:
        _CACHE["nc"] = build()
    nc = _CACHE["nc"]
    in_maps = host_prep(x, W, mins, maxs)
    res = run_bass_kernel_spmd(nc, in_maps, core_ids=list(range(N_CORES)),
                               trace=trace, **trace_kw)
    out = res.results[0]["out"].reshape(NUM_PROJ, PROJ_DIM, BINS)
    return np.asarray(out, dtype=np.float32), res


def kernel(x, W, mins, maxs, num_of_projection=NUM_PROJ, bins=BINS):
    assert int(num_of_projection) == NUM_PROJ and int(bins) == BINS
    out, _ = run(x, W, mins, maxs, trace=False)
    return out


# revision 9
# speedup vs baseline: 1.9702x; 1.2040x over previous
"""Trainium2 Bass kernel for nn_RandProjector (histogram_binning).

Computes, for x [16384, 1024] and W [6400, 1024]:
    proj = x @ W.T                      # [S, D] -- never materialized in HBM
    per-column 20-bin histogram of proj (torch.histc semantics with
    mins/maxs as ranges), reshaped [100, 64, 20], L2-normalized over bins.

Strategy (8 NeuronCores, data-parallel over S):
  - Host pre-transposes x and W (plain multi-queue DMA loads, no xbar
    transpose serialization) and scales W rows by bins/width_d so the
    fp16 matmul accumulates proj' = proj * bins/width in PSUM fp32.
  - ScalarE stages each PSUM tile to SBUF fp16 with a per-partition bias
    (Identity activation): u = proj' - min*bins/width in [0, 20].  This
    frees the PSUM slot after ~2us so the PE never stalls on compare
    progress, and makes all 19 bin edges integer constants.
  - cdf_b = #(u >= b) via fused compare+accumulate.  The DVE accumulate
    path runs at 1x regardless of dtype (HW-measured), so VectorE uses a
    custom dual-edge DVE op (registered below):
        out = (x >= s0) + (x >= s1) * 4096 ; accum_out = sum(out)
    packing two exact counts (<= 2048 each) into one fp32 accumulator --
    14 edges in 7 passes, paired (b, b+7) so the unpacked lo/hi counts
    land in contiguous cdf column ranges.  ScalarE handles the remaining
    5 edges via Sign activation (+accum).
  - Bin counts are cdf differences; bin 0 uses the constant shard total.
  - Histogram pieces AllReduce across the 8 cores as soon as their tiles
    finish (overlapping compute); the last piece is small so the
    trailing collective is short.  L2-normalize on device.
"""

import sys

if "/opt/trn_rl_repo" not in sys.path:
    sys.path.insert(0, "/opt/trn_rl_repo")

import numpy as np

S, IN_DIM = 16384, 1024
NUM_PROJ, PROJ_DIM, BINS = 100, 64, 20
D = NUM_PROJ * PROJ_DIM          # 6400
N_CORES = 8
S_SHARD = S // N_CORES           # 2048
NE = BINS - 1                    # 19 interior edges (b = 1..19)
NP2 = 7                          # dual-edge passes on VectorE
NV = 2 * NP2                     # edges 1..14 on VectorE (pairs (b, b+7))
NA = NE - NV                     # edges 15..19 on ScalarE
PK = 4096.0                      # dual-edge packing factor

_CACHE = {}


def _cdf2_ref(in0, in1, s0, s1, imm2):
    x = in0.astype(np.float32)
    g = (x >= s0).astype(np.float32) + (x >= s1).astype(np.float32) * imm2
    return g, g.reshape(g.shape[0], -1).sum(axis=-1, keepdims=True)


def _register_cdf2():
    """Register the dual-edge compare+accumulate custom DVE op (idempotent)."""
    from operator import add

    from concourse import dve_ops
    from concourse.dve_spec import C0, C1, C2, Spec, Src0, Zero, lower
    from concourse.dve_uop import DveOpSpec

    if "CDF2_ANT" in dve_ops._SUB_OPCODE_FOR_NAME:
        return next(op for op in dve_ops.OPS if op.name == "CDF2_ANT")
    spec = Spec(
        body=(Src0 >= C0) + (Src0 >= C1) * C2,
        accum=add,
        accum_init=Zero,
        reference=_cdf2_ref,
    )
    row = max(dve_ops._SUB_OPCODE_FOR_NAME.values()) + 1
    assert row < 0x20
    sha = {}
    for ver in ("v3", "v4"):
        tmp = DveOpSpec(name="CDF2_ANT", opcode=row,
                        uops=lower(spec, ver=ver), rd1_en=False)
        sha[ver] = tmp.sha(ver)
    op = dve_ops.DveOp("CDF2_ANT", spec, subdim=False, uops_sha=sha)
    dve_ops.OPS.append(op)
    dve_ops.CUSTOM_DVE_SPECS[op.name] = op.spec
    dve_ops._SUB_OPCODE_FOR_NAME[op.name] = row
    return op


def build(s_shard=S_SHARD, d=D, in_dim=IN_DIM, n_cores=N_CORES, debug=False):
    import concourse.bacc as bacc
    import concourse.bass as bass
    from concourse import mybir
    from concourse.tile import TileContext

    cdf2 = _register_cdf2()

    f32 = mybir.dt.float32
    f16 = mybir.dt.float16
    i32 = mybir.dt.int32
    nt = d // 128                # 50
    kc_n = in_dim // 128         # 8
    chw = min(512, s_shard)      # matmul moving-operand width
    nch_n = s_shard // chw       # 4

    nc = bacc.Bacc("TRN2", target_bir_lowering=False, debug=False,
                   num_devices=n_cores)

    xT_d = nc.dram_tensor("xT16", [kc_n, 128, s_shard], f16,
                          kind="ExternalInput")
    wT_d = nc.dram_tensor("wT16", [kc_n, 128, d], f16, kind="ExternalInput")
    bias_d = nc.dram_tensor("bias", [128, nt], f32, kind="ExternalInput")
    out_d = nc.dram_tensor("out", [d, BINS], f32, kind="ExternalOutput")
    # pieces of the histogram all-reduce independently, overlapping compute;
    # the last piece is small so the trailing collective is short
    pieces = [(0, 22), (22, 42), (42, 48), (48, nt)]
    cc_ins, cc_outs = [], []
    for i, (t0, t1) in enumerate(pieces):
        cc_ins.append(nc.dram_tensor(f"cc_in{i}", [128, (t1 - t0) * BINS], f32))
        cc_outs.append(nc.dram_tensor(f"cc_out{i}", [128, (t1 - t0) * BINS],
                                      f32, addr_space="Shared"))
    if debug:
        dbg_hist = nc.dram_tensor("dbg_hist", [128, nt, BINS], f32,
                                  kind="ExternalOutput")
        dbg_cdf = nc.dram_tensor("dbg_cdf", [128, nt, BINS + 1], f32,
                                 kind="ExternalOutput")

    with TileContext(nc) as tc:
        with (
            tc.tile_pool(name="singles", bufs=1) as singles,
            tc.tile_pool(name="sp_pool", bufs=3) as sp_pool,
            tc.tile_pool(name="ps_p", bufs=2, space="PSUM") as ps_p,
        ):
            bias_sb = singles.tile([128, nt], f32)
            nc.sync.dma_start(out=bias_sb, in_=bias_d[:, :])

            # ScalarE Sign biases (-b for b = NV+1..19), one column each
            negb = singles.tile([128, NA], f32)
            for j in range(NA):
                nc.vector.memset(negb[:, j:j + 1], float(-(NV + 1 + j)))

            trash_v = singles.tile([128, s_shard], f32)
            trash_a = singles.tile([128, s_shard], f16)

            # per-engine cdf accumulators (separate tiles so Tile never
            # serializes VectorE against ScalarE on writes)
            accp = singles.tile([128, nt, NP2], f32)   # packed dual counts
            acc_a = singles.tile([128, nt, NA], f32)

            # ---- Phase 0: plain DMA of pre-transposed x shard and W ----
            xT = singles.tile([128, kc_n, s_shard], f16)
            wT = singles.tile([128, kc_n, d], f16)
            for nch in range(nch_n):
                sl = slice(nch * chw, (nch + 1) * chw)
                for kc in range(kc_n):
                    nc.sync.dma_start(out=xT[:, kc, sl], in_=xT_d[kc, :, sl])
            d_bounds = [0]
            while d_bounds[-1] < d:
                nxt = 256 if d_bounds[-1] == 0 else 800
                d_bounds.append(min(d_bounds[-1] + nxt, d))
            for d0, d1 in zip(d_bounds[:-1], d_bounds[1:]):
                for kc in range(kc_n):
                    nc.sync.dma_start(out=wT[:, kc, d0:d1],
                                      in_=wT_d[kc, :, d0:d1])

            # normalization / unpack scratch
            cdfx = singles.tile([128, nt, BINS + 1], f32)
            nc.vector.memset(cdfx[:, :, 0:1], float(s_shard))
            nc.vector.memset(cdfx[:, :, BINS:BINS + 1], 0.0)
            ip_t = singles.tile([128, nt, NP2], i32)
            ip2_t = singles.tile([128, nt, NP2], i32)
            hi_t = singles.tile([128, nt, NP2], f32)
            hist = singles.tile([128, nt, BINS], f32)
            hsum = singles.tile([128, nt, BINS], f32)
            sq = singles.tile([128, nt, BINS], f32)
            n2 = singles.tile([128, nt], f32)
            y_t = singles.tile([128, nt], f32)
            iy = singles.tile([128, nt], f32)
            a_t = singles.tile([128, nt], f32)
            b_t = singles.tile([128, nt], f32)
            r_t = singles.tile([128, nt], f32)
            outn = singles.tile([128, nt, BINS], f32)
            out_v = out_d[:, :].rearrange("(t p) b -> p t b", p=128)

            def emit_cc(hi):
                """Unpack cdf partials for tau in [t0, t1) and kick off the
                cross-core all-reduce (runs on DMA/CC queues in background)."""
                t0, t1 = pieces[hi]
                sl = slice(t0, t1)
                # unpack: lo = accp & 0xFFF (exact: accp is integer-valued
                # fp32 < 2^23, i32 convert exact), hi = (accp - lo) / 4096
                nc.vector.tensor_copy(ip_t[:, sl], accp[:, sl])
                nc.vector.tensor_scalar(ip2_t[:, sl], ip_t[:, sl], 0xFFF,
                                        None, op0=mybir.AluOpType.bitwise_and)
                nc.vector.tensor_copy(cdfx[:, sl, 1:1 + NP2], ip2_t[:, sl])
                nc.vector.tensor_tensor(out=hi_t[:, sl], in0=accp[:, sl],
                                        in1=cdfx[:, sl, 1:1 + NP2],
                                        op=mybir.AluOpType.subtract)
                nc.vector.tensor_scalar(cdfx[:, sl, 1 + NP2:1 + NV],
                                        hi_t[:, sl], 1.0 / PK, None,
                                        op0=mybir.AluOpType.mult)
                # ScalarE counts are sums of sign in {-1,0,1}:
                # cdf = 0.5*sgn + N/2
                nc.vector.tensor_scalar(
                    cdfx[:, sl, 1 + NV:BINS], acc_a[:, sl],
                    0.5, float(s_shard) / 2,
                    op0=mybir.AluOpType.mult, op1=mybir.AluOpType.add)
                nc.vector.tensor_tensor(
                    out=hist[:, sl], in0=cdfx[:, sl, 0:BINS],
                    in1=cdfx[:, sl, 1:BINS + 1],
                    op=mybir.AluOpType.subtract)
                if debug and hi == len(pieces) - 1:
                    nc.sync.dma_start(out=dbg_hist[:, :, :], in_=hist)
                    nc.sync.dma_start(out=dbg_cdf[:, :, :], in_=cdfx)
                nc.sync.dma_start(
                    out=cc_ins[hi][:, :],
                    in_=hist[:, sl].rearrange("p a b -> p (a b)"))
                nc.gpsimd.collective_compute(
                    "AllReduce",
                    mybir.AluOpType.add,
                    replica_groups=[list(range(n_cores))],
                    ins=[cc_ins[hi][:, :]],
                    outs=[cc_outs[hi][:, :]],
                )
                nc.sync.dma_start(
                    out=hsum[:, sl].rearrange("p a b -> p (a b)"),
                    in_=cc_outs[hi][:, :])

            hsum_g = singles.tile([128, nt, BINS], f32)

            def emit_norm(t0, t1, guard):
                """L2-normalize the summed histogram for tau in [t0, t1) and
                write the output slice. When `guard`, route hsum through a
                no-op add of last-tile accumulator data so the scheduler's
                cost model places the chain after the final tile -- the
                collective is then long finished and no engine FIFO stalls
                on it."""
                sl = slice(t0, t1)
                w = t1 - t0
                if guard:
                    g_ap = acc_a[:, nt - 1, NA - 1:NA]
                    g_b = bass.AP(tensor=g_ap.tensor, offset=g_ap.offset,
                                  ap=[g_ap.ap[0], [0, w], [0, BINS]])
                    nc.vector.scalar_tensor_tensor(
                        out=hsum_g[:, sl], in0=g_b, scalar=0.0,
                        in1=hsum[:, sl],
                        op0=mybir.AluOpType.mult, op1=mybir.AluOpType.add)
                    h_in = hsum_g
                else:
                    h_in = hsum
                nc.vector.tensor_tensor(out=sq[:, sl], in0=h_in[:, sl],
                                        in1=h_in[:, sl],
                                        op=mybir.AluOpType.mult)
                nc.vector.tensor_reduce(out=n2[:, sl], in_=sq[:, sl],
                                        axis=mybir.AxisListType.X,
                                        op=mybir.AluOpType.add)
                nc.scalar.sqrt(y_t[:, sl], n2[:, sl])
                nc.vector.reciprocal(iy[:, sl], y_t[:, sl])
                # one Newton step for rsqrt: r = iy * (1.5 - 0.5*n2*iy^2)
                nc.vector.tensor_tensor(out=a_t[:, sl], in0=iy[:, sl],
                                        in1=iy[:, sl],
                                        op=mybir.AluOpType.mult)
                nc.vector.tensor_tensor(out=b_t[:, sl], in0=a_t[:, sl],
                                        in1=n2[:, sl],
                                        op=mybir.AluOpType.mult)
                nc.vector.tensor_scalar(b_t[:, sl], b_t[:, sl], -0.5, 1.5,
                                        op0=mybir.AluOpType.mult,
                                        op1=mybir.AluOpType.add)
                nc.vector.tensor_tensor(out=r_t[:, sl], in0=iy[:, sl],
                                        in1=b_t[:, sl],
                                        op=mybir.AluOpType.mult)
                r_ap = r_t[:, sl]
                r_b = bass.AP(tensor=r_ap.tensor, offset=r_ap.offset,
                              ap=[r_ap.ap[0], r_ap.ap[1], [0, BINS]])
                nc.vector.tensor_tensor(out=outn[:, sl], in0=h_in[:, sl],
                                        in1=r_b, op=mybir.AluOpType.mult)
                nc.sync.dma_start(out=out_v[:, sl], in_=outn[:, sl])

            # ---- Phase 1: d-tiles ----
            for tau in range(nt):
                pp = ps_p.tile([128, s_shard], f32)
                for nch in range(nch_n):
                    for kc in range(kc_n):
                        nc.tensor.matmul(
                            pp[:, nch * chw:(nch + 1) * chw],
                            lhsT=wT[:, kc, tau * 128:(tau + 1) * 128],
                            rhs=xT[:, kc, nch * chw:(nch + 1) * chw],
                            start=(kc == 0),
                            stop=(kc == kc_n - 1),
                        )
                # Stage PSUM -> SBUF fp16 once, adding the per-dim bias:
                # u = proj' - min * bins/width  in [0, bins].  Frees the
                # PSUM slot after ~2us so the PE is never gated on compares.
                sp = sp_pool.tile([128, s_shard], f16)
                nc.scalar.activation(
                    out=sp, in_=pp,
                    func=mybir.ActivationFunctionType.Identity,
                    bias=bias_sb[:, tau:tau + 1], scale=1.0)
                # VectorE: edges b and b+7 per dual pass (integer consts)
                for j in range(NP2):
                    nc.vector._custom_dve(
                        cdf2,
                        out=trash_v,
                        in0=sp,
                        s0=float(1 + j),
                        s1=float(1 + j + NP2),
                        imm2=PK,
                        accum_out=accp[:, tau, j:j + 1],
                    )
                # ScalarE: edges b = NV+1 .. 19 via Sign(u - b)
                for j in range(NA):
                    nc.scalar.activation(
                        out=trash_a,
                        in_=sp,
                        func=mybir.ActivationFunctionType.Sign,
                        bias=negb[:, j:j + 1],
                        scale=1.0,
                        accum_out=acc_a[:, tau, j:j + 1],
                    )
                for hi, (t0, t1) in enumerate(pieces):
                    if tau == t1 - 1:
                        emit_cc(hi)
            # all normalization at the end: earlier pieces' collectives have
            # long completed (guarded so the scheduler knows); only the last
            # piece's collective is actually waited on
            emit_norm(0, pieces[-2][1], guard=True)
            emit_norm(pieces[-1][0], pieces[-1][1], guard=False)

    nc.compile()
    return nc


def host_prep(x, W, mins, maxs, s_shard=S_SHARD, n_cores=N_CORES):
    d = W.shape[0]
    nt = d // 128
    kc_n = IN_DIM // 128
    mins64 = np.asarray(mins, dtype=np.float64)
    maxs64 = np.asarray(maxs, dtype=np.float64)
    scale = float(BINS) / (maxs64 - mins64)                        # [d]
    # W rows scaled so the matmul yields proj' = proj * bins/width
    Ws = np.asarray(W, dtype=np.float64) * scale[:, None]
    w16T = np.ascontiguousarray(Ws.T.astype(np.float16))           # [in, d]
    w16T = w16T.reshape(kc_n, 128, d)
    x16T = np.ascontiguousarray(np.asarray(x, dtype=np.float16).T)  # [in, S]
    x16T = x16T.reshape(kc_n, 128, S)
    bias = (-mins64 * scale).astype(np.float32)                    # [d]
    bias_l = np.ascontiguousarray(bias.reshape(nt, 128).T)         # [128, nt]
    in_maps = []
    for i in range(n_cores):
        in_maps.append({
            "xT16": np.ascontiguousarray(
                x16T[:, :, i * s_shard:(i + 1) * s_shard]),
            "wT16": w16T,
            "bias": bias_l,
        })
    return in_maps


def run(x, W, mins, maxs, trace=False, **trace_kw):
    """Returns (output [100, 64, 20] f32, BassKernelResults)."""
    from concourse.bass_utils import run_bass_kernel_spmd

    if "nc" not in _CACHE:
        _CACHE["nc"] = build()
    nc = _CACHE["nc"]
    in_maps = host_prep(x, W, mins, maxs)
    res = run_bass_kernel_spmd(nc, in_maps, core_ids=list(range(N_CORES)),
                               trace=trace, **trace_kw)
    out = res.results[0]["out"].reshape(NUM_PROJ, PROJ_DIM, BINS)
    return np.asarray(out, dtype=np.float32), res


def kernel(x, W, mins, maxs, num_of_projection=NUM_PROJ, bins=BINS):
    assert int(num_of_projection) == NUM_PROJ and int(bins) == BINS
    out, _ = run(x, W, mins, maxs, trace=False)
    return out


# revision 12
# speedup vs baseline: 2.0370x; 1.0339x over previous
"""Trainium2 Bass kernel for nn_RandProjector (histogram_binning).

Computes, for x [16384, 1024] and W [6400, 1024]:
    proj = x @ W.T                      # [S, D] -- never materialized in HBM
    per-column 20-bin histogram of proj (torch.histc semantics with
    mins/maxs as ranges), reshaped [100, 64, 20], L2-normalized over bins.

Strategy (8 NeuronCores, data-parallel over S):
  - Host pre-transposes x and W (plain multi-queue DMA loads, no xbar
    transpose serialization) and scales W rows by bins/width_d so the
    fp16 matmul accumulates proj' = proj * bins/width in PSUM fp32.
  - ScalarE stages each PSUM tile to SBUF fp16 with a per-partition bias
    (Identity activation): u = proj' - min*bins/width in [0, 20].  This
    frees the PSUM slot after ~2us so the PE never stalls on compare
    progress, and makes all 19 bin edges integer constants.
  - cdf_b = #(u >= b) via fused compare+accumulate.  The DVE accumulate
    path runs at 1x regardless of dtype (HW-measured), so VectorE uses a
    custom dual-edge DVE op (registered below):
        out = (x >= s0) + (x >= s1) * 4096 ; accum_out = sum(out)
    packing two exact counts (<= 2048 each) into one fp32 accumulator --
    14 edges in 7 passes, paired (b, b+7) so the unpacked lo/hi counts
    land in contiguous cdf column ranges.  ScalarE handles the remaining
    5 edges via Sign activation (+accum).
  - Bin counts are cdf differences; bin 0 uses the constant shard total.
  - Histogram pieces AllReduce across the 8 cores as soon as their tiles
    finish (overlapping compute); the last piece is small so the
    trailing collective is short.  L2-normalize on device.
"""

import sys

if "/opt/trn_rl_repo" not in sys.path:
    sys.path.insert(0, "/opt/trn_rl_repo")

import numpy as np

S, IN_DIM = 16384, 1024
NUM_PROJ, PROJ_DIM, BINS = 100, 64, 20
D = NUM_PROJ * PROJ_DIM          # 6400
N_CORES = 8
S_SHARD = S // N_CORES           # 2048
NE = BINS - 1                    # 19 interior edges (b = 1..19)
NP2 = 7                          # dual-edge passes on VectorE
NV = 2 * NP2                     # edges 1..14 on VectorE (pairs (b, b+7))
NA = NE - NV                     # edges 15..19 on ScalarE
PK = 4096.0                      # dual-edge packing factor

_CACHE = {}


def _cdf2_ref(in0, in1, s0, s1, imm2):
    x = in0.astype(np.float32)
    g = (x >= s0).astype(np.float32) + (x >= s1).astype(np.float32) * imm2
    return g, g.reshape(g.shape[0], -1).sum(axis=-1, keepdims=True)


def _register_cdf2():
    """Register the dual-edge compare+accumulate custom DVE op (idempotent)."""
    from operator import add

    from concourse import dve_ops
    from concourse.dve_spec import C0, C1, C2, Spec, Src0, Zero, lower
    from concourse.dve_uop import DveOpSpec

    if "CDF2_ANT" in dve_ops._SUB_OPCODE_FOR_NAME:
        return next(op for op in dve_ops.OPS if op.name == "CDF2_ANT")
    spec = Spec(
        body=(Src0 >= C0) + (Src0 >= C1) * C2,
        accum=add,
        accum_init=Zero,
        reference=_cdf2_ref,
    )
    row = max(dve_ops._SUB_OPCODE_FOR_NAME.values()) + 1
    assert row < 0x20
    sha = {}
    for ver in ("v3", "v4"):
        tmp = DveOpSpec(name="CDF2_ANT", opcode=row,
                        uops=lower(spec, ver=ver), rd1_en=False)
        sha[ver] = tmp.sha(ver)
    op = dve_ops.DveOp("CDF2_ANT", spec, subdim=False, uops_sha=sha)
    dve_ops.OPS.append(op)
    dve_ops.CUSTOM_DVE_SPECS[op.name] = op.spec
    dve_ops._SUB_OPCODE_FOR_NAME[op.name] = row
    return op


def build(s_shard=S_SHARD, d=D, in_dim=IN_DIM, n_cores=N_CORES, debug=False):
    import concourse.bacc as bacc
    import concourse.bass as bass
    from concourse import mybir
    from concourse.tile import TileContext

    cdf2 = _register_cdf2()

    f32 = mybir.dt.float32
    f16 = mybir.dt.float16
    i32 = mybir.dt.int32
    nt = d // 128                # 50
    kc_n = in_dim // 128         # 8
    chw = min(512, s_shard)      # matmul moving-operand width
    nch_n = s_shard // chw       # 4

    nc = bacc.Bacc("TRN2", target_bir_lowering=False, debug=False,
                   num_devices=n_cores)

    xT_d = nc.dram_tensor("xT16", [kc_n, 128, s_shard], f16,
                          kind="ExternalInput")
    wT_d = nc.dram_tensor("wT16", [kc_n, 128, d], f16, kind="ExternalInput")
    bias_d = nc.dram_tensor("bias", [128, nt], f32, kind="ExternalInput")
    out_d = nc.dram_tensor("out", [d, BINS], f32, kind="ExternalOutput")
    # pieces of the histogram all-reduce independently, overlapping compute;
    # the last piece is small so the trailing collective is short
    pieces = [(0, 22), (22, 42), (42, 47), (47, 49), (49, nt)]
    cc_ins, cc_outs = [], []
    for i, (t0, t1) in enumerate(pieces):
        cc_ins.append(nc.dram_tensor(f"cc_in{i}", [128, (t1 - t0) * BINS], f32))
        cc_outs.append(nc.dram_tensor(f"cc_out{i}", [128, (t1 - t0) * BINS],
                                      f32, addr_space="Shared"))
    if debug:
        dbg_hist = nc.dram_tensor("dbg_hist", [128, nt, BINS], f32,
                                  kind="ExternalOutput")
        dbg_cdf = nc.dram_tensor("dbg_cdf", [128, nt, BINS + 1], f32,
                                 kind="ExternalOutput")

    with TileContext(nc) as tc:
        with (
            tc.tile_pool(name="singles", bufs=1) as singles,
            tc.tile_pool(name="sp_pool", bufs=3) as sp_pool,
            tc.tile_pool(name="ps_p", bufs=2, space="PSUM") as ps_p,
        ):
            bias_sb = singles.tile([128, nt], f32)
            nc.sync.dma_start(out=bias_sb, in_=bias_d[:, :])

            # ScalarE Sign biases (-b for b = NV+1..19), one column each
            negb = singles.tile([128, NA], f32)
            for j in range(NA):
                nc.vector.memset(negb[:, j:j + 1], float(-(NV + 1 + j)))

            trash_v = singles.tile([128, s_shard], f32)
            trash_a = singles.tile([128, s_shard], f16)

            # per-engine cdf accumulators (separate tiles so Tile never
            # serializes VectorE against ScalarE on writes)
            accp = singles.tile([128, nt, NP2], f32)   # packed dual counts
            acc_a = singles.tile([128, nt, NA], f32)

            # ---- Phase 0: plain DMA of pre-transposed x shard and W ----
            xT = singles.tile([128, kc_n, s_shard], f16)
            wT = singles.tile([128, kc_n, d], f16)
            for nch in range(nch_n):
                sl = slice(nch * chw, (nch + 1) * chw)
                for kc in range(kc_n):
                    nc.sync.dma_start(out=xT[:, kc, sl], in_=xT_d[kc, :, sl])
            d_bounds = [0]
            while d_bounds[-1] < d:
                nxt = 256 if d_bounds[-1] == 0 else 800
                d_bounds.append(min(d_bounds[-1] + nxt, d))
            for d0, d1 in zip(d_bounds[:-1], d_bounds[1:]):
                for kc in range(kc_n):
                    nc.sync.dma_start(out=wT[:, kc, d0:d1],
                                      in_=wT_d[kc, :, d0:d1])

            # normalization / unpack scratch
            cdfx = singles.tile([128, nt, BINS + 1], f32)
            nc.vector.memset(cdfx[:, :, 0:1], float(s_shard))
            nc.vector.memset(cdfx[:, :, BINS:BINS + 1], 0.0)
            ip_t = singles.tile([128, nt, NP2], i32)
            ip2_t = singles.tile([128, nt, NP2], i32)
            hi_t = singles.tile([128, nt, NP2], f32)
            hist = singles.tile([128, nt, BINS], f32)
            hsum = singles.tile([128, nt, BINS], f32)
            sq = singles.tile([128, nt, BINS], f32)
            n2 = singles.tile([128, nt], f32)
            y_t = singles.tile([128, nt], f32)
            iy = singles.tile([128, nt], f32)
            a_t = singles.tile([128, nt], f32)
            b_t = singles.tile([128, nt], f32)
            r_t = singles.tile([128, nt], f32)
            outn = singles.tile([128, nt, BINS], f32)
            out_v = out_d[:, :].rearrange("(t p) b -> p t b", p=128)

            def emit_cc(hi):
                """Unpack cdf partials for tau in [t0, t1) and kick off the
                cross-core all-reduce (runs on DMA/CC queues in background)."""
                t0, t1 = pieces[hi]
                sl = slice(t0, t1)
                # unpack: lo = accp & 0xFFF (exact: accp is integer-valued
                # fp32 < 2^23, i32 convert exact), hi = (accp - lo) / 4096
                nc.vector.tensor_copy(ip_t[:, sl], accp[:, sl])
                nc.vector.tensor_scalar(ip2_t[:, sl], ip_t[:, sl], 0xFFF,
                                        None, op0=mybir.AluOpType.bitwise_and)
                nc.vector.tensor_copy(cdfx[:, sl, 1:1 + NP2], ip2_t[:, sl])
                nc.vector.tensor_tensor(out=hi_t[:, sl], in0=accp[:, sl],
                                        in1=cdfx[:, sl, 1:1 + NP2],
                                        op=mybir.AluOpType.subtract)
                nc.vector.tensor_scalar(cdfx[:, sl, 1 + NP2:1 + NV],
                                        hi_t[:, sl], 1.0 / PK, None,
                                        op0=mybir.AluOpType.mult)
                # ScalarE counts are sums of sign in {-1,0,1}:
                # cdf = 0.5*sgn + N/2
                nc.vector.tensor_scalar(
                    cdfx[:, sl, 1 + NV:BINS], acc_a[:, sl],
                    0.5, float(s_shard) / 2,
                    op0=mybir.AluOpType.mult, op1=mybir.AluOpType.add)
                nc.vector.tensor_tensor(
                    out=hist[:, sl], in0=cdfx[:, sl, 0:BINS],
                    in1=cdfx[:, sl, 1:BINS + 1],
                    op=mybir.AluOpType.subtract)
                if debug and hi == len(pieces) - 1:
                    nc.sync.dma_start(out=dbg_hist[:, :, :], in_=hist)
                    nc.sync.dma_start(out=dbg_cdf[:, :, :], in_=cdfx)
                nc.sync.dma_start(
                    out=cc_ins[hi][:, :],
                    in_=hist[:, sl].rearrange("p a b -> p (a b)"))
                nc.gpsimd.collective_compute(
                    "AllReduce",
                    mybir.AluOpType.add,
                    replica_groups=[list(range(n_cores))],
                    ins=[cc_ins[hi][:, :]],
                    outs=[cc_outs[hi][:, :]],
                )
                nc.sync.dma_start(
                    out=hsum[:, sl].rearrange("p a b -> p (a b)"),
                    in_=cc_outs[hi][:, :])

            hsum_g = singles.tile([128, nt, BINS], f32)

            def emit_norm(t0, t1, anchor=None):
                """L2-normalize the summed histogram for tau in [t0, t1) and
                write the output slice. When `anchor` is a tile index, route
                hsum through a no-op add of that tile's accumulator data so
                the scheduler's cost model places the chain after that tile's
                compares -- the collective is then long finished and no
                engine FIFO stalls on it."""
                sl = slice(t0, t1)
                w = t1 - t0
                if anchor is not None:
                    g_ap = acc_a[:, anchor, NA - 1:NA]
                    g_b = bass.AP(tensor=g_ap.tensor, offset=g_ap.offset,
                                  ap=[g_ap.ap[0], [0, w], [0, BINS]])
                    nc.vector.scalar_tensor_tensor(
                        out=hsum_g[:, sl], in0=g_b, scalar=0.0,
                        in1=hsum[:, sl],
                        op0=mybir.AluOpType.mult, op1=mybir.AluOpType.add)
                    h_in = hsum_g
                else:
                    h_in = hsum
                nc.vector.tensor_tensor(out=sq[:, sl], in0=h_in[:, sl],
                                        in1=h_in[:, sl],
                                        op=mybir.AluOpType.mult)
                nc.vector.tensor_reduce(out=n2[:, sl], in_=sq[:, sl],
                                        axis=mybir.AxisListType.X,
                                        op=mybir.AluOpType.add)
                nc.scalar.sqrt(y_t[:, sl], n2[:, sl])
                nc.vector.reciprocal(iy[:, sl], y_t[:, sl])
                # one Newton step for rsqrt: r = iy * (1.5 - 0.5*n2*iy^2)
                nc.vector.tensor_tensor(out=a_t[:, sl], in0=iy[:, sl],
                                        in1=iy[:, sl],
                                        op=mybir.AluOpType.mult)
                nc.vector.tensor_tensor(out=b_t[:, sl], in0=a_t[:, sl],
                                        in1=n2[:, sl],
                                        op=mybir.AluOpType.mult)
                nc.vector.tensor_scalar(b_t[:, sl], b_t[:, sl], -0.5, 1.5,
                                        op0=mybir.AluOpType.mult,
                                        op1=mybir.AluOpType.add)
                nc.vector.tensor_tensor(out=r_t[:, sl], in0=iy[:, sl],
                                        in1=b_t[:, sl],
                                        op=mybir.AluOpType.mult)
                r_ap = r_t[:, sl]
                r_b = bass.AP(tensor=r_ap.tensor, offset=r_ap.offset,
                              ap=[r_ap.ap[0], r_ap.ap[1], [0, BINS]])
                nc.vector.tensor_tensor(out=outn[:, sl], in0=h_in[:, sl],
                                        in1=r_b, op=mybir.AluOpType.mult)
                nc.sync.dma_start(out=out_v[:, sl], in_=outn[:, sl])

            # ---- Phase 1: d-tiles ----
            for tau in range(nt):
                pp = ps_p.tile([128, s_shard], f32)
                for nch in range(nch_n):
                    for kc in range(kc_n):
                        nc.tensor.matmul(
                            pp[:, nch * chw:(nch + 1) * chw],
                            lhsT=wT[:, kc, tau * 128:(tau + 1) * 128],
                            rhs=xT[:, kc, nch * chw:(nch + 1) * chw],
                            start=(kc == 0),
                            stop=(kc == kc_n - 1),
                        )
                # Stage PSUM -> SBUF fp16 once, adding the per-dim bias:
                # u = proj' - min * bins/width  in [0, bins].  Frees the
                # PSUM slot after ~2us so the PE is never gated on compares.
                sp = sp_pool.tile([128, s_shard], f16)
                nc.scalar.activation(
                    out=sp, in_=pp,
                    func=mybir.ActivationFunctionType.Identity,
                    bias=bias_sb[:, tau:tau + 1], scale=1.0)
                # VectorE: edges b and b+7 per dual pass (integer consts)
                for j in range(NP2):
                    nc.vector._custom_dve(
                        cdf2,
                        out=trash_v,
                        in0=sp,
                        s0=float(1 + j),
                        s1=float(1 + j + NP2),
                        imm2=PK,
                        accum_out=accp[:, tau, j:j + 1],
                    )
                # ScalarE: edges b = NV+1 .. 19 via Sign(u - b)
                for j in range(NA):
                    nc.scalar.activation(
                        out=trash_a,
                        in_=sp,
                        func=mybir.ActivationFunctionType.Sign,
                        bias=negb[:, j:j + 1],
                        scale=1.0,
                        accum_out=acc_a[:, tau, j:j + 1],
                    )
                for hi, (t0, t1) in enumerate(pieces):
                    if tau == t1 - 1:
                        emit_cc(hi)
            # normalization staged so the bulk runs during the last tiles'
            # compute (its collectives have long completed); only the last
            # piece's collective is actually waited on at the end
            emit_norm(0, 42, anchor=46)
            emit_norm(42, pieces[-2][1], anchor=nt - 1)
            emit_norm(pieces[-1][0], pieces[-1][1])

    nc.compile()
    return nc


def host_prep(x, W, mins, maxs, s_shard=S_SHARD, n_cores=N_CORES):
    d = W.shape[0]
    nt = d // 128
    kc_n = IN_DIM // 128
    mins64 = np.asarray(mins, dtype=np.float64)
    maxs64 = np.asarray(maxs, dtype=np.float64)
    scale = float(BINS) / (maxs64 - mins64)                        # [d]
    # W rows scaled so the matmul yields proj' = proj * bins/width
    Ws = np.asarray(W, dtype=np.float64) * scale[:, None]
    w16T = np.ascontiguousarray(Ws.T.astype(np.float16))           # [in, d]
    w16T = w16T.reshape(kc_n, 128, d)
    x16T = np.ascontiguousarray(np.asarray(x, dtype=np.float16).T)  # [in, S]
    x16T = x16T.reshape(kc_n, 128, S)
    bias = (-mins64 * scale).astype(np.float32)                    # [d]
    bias_l = np.ascontiguousarray(bias.reshape(nt, 128).T)         # [128, nt]
    in_maps = []
    for i in range(n_cores):
        in_maps.append({
            "xT16": np.ascontiguousarray(
                x16T[:, :, i * s_shard:(i + 1) * s_shard]),
            "wT16": w16T,
            "bias": bias_l,
        })
    return in_maps


def run(x, W, mins, maxs, trace=False, **trace_kw):
    """Returns (output [100, 64, 20] f32, BassKernelResults)."""
    from concourse.bass_utils import run_bass_kernel_spmd

    if "nc" not in _CACHE:
        _CACHE["nc"] = build()
    nc = _CACHE["nc"]
    in_maps = host_prep(x, W, mins, maxs)
    res = run_bass_kernel_spmd(nc, in_maps, core_ids=list(range(N_CORES)),
                               trace=trace, **trace_kw)
    out = res.results[0]["out"].reshape(NUM_PROJ, PROJ_DIM, BINS)
    return np.asarray(out, dtype=np.float32), res


def kernel(x, W, mins, maxs, num_of_projection=NUM_PROJ, bins=BINS):
    assert int(num_of_projection) == NUM_PROJ and int(bins) == BINS
    out, _ = run(x, W, mins, maxs, trace=False)
    return out


# revision 13
# speedup vs baseline: 2.0393x; 1.0011x over previous
"""Trainium2 Bass kernel for nn_RandProjector (histogram_binning).

Computes, for x [16384, 1024] and W [6400, 1024]:
    proj = x @ W.T                      # [S, D] -- never materialized in HBM
    per-column 20-bin histogram of proj (torch.histc semantics with
    mins/maxs as ranges), reshaped [100, 64, 20], L2-normalized over bins.

Strategy (8 NeuronCores, data-parallel over S):
  - Host pre-transposes x and W (plain multi-queue DMA loads, no xbar
    transpose serialization) and scales W rows by bins/width_d so the
    fp16 matmul accumulates proj' = proj * bins/width in PSUM fp32.
  - ScalarE stages each PSUM tile to SBUF fp16 with a per-partition bias
    (Identity activation): u = proj' - min*bins/width in [0, 20].  This
    frees the PSUM slot after ~2us so the PE never stalls on compare
    progress, and makes all 19 bin edges integer constants.
  - cdf_b = #(u >= b) via fused compare+accumulate.  The DVE accumulate
    path runs at 1x regardless of dtype (HW-measured), so VectorE uses a
    custom dual-edge DVE op (registered below):
        out = (x >= s0) + (x >= s1) * 4096 ; accum_out = sum(out)
    packing two exact counts (<= 2048 each) into one fp32 accumulator --
    14 edges in 7 passes, paired (b, b+7) so the unpacked lo/hi counts
    land in contiguous cdf column ranges.  ScalarE handles the remaining
    5 edges via Sign activation (+accum).
  - Bin counts are cdf differences; bin 0 uses the constant shard total.
  - Histogram pieces AllReduce across the 8 cores as soon as their tiles
    finish (overlapping compute); the last piece is small so the
    trailing collective is short.  L2-normalize on device.
"""

import sys

if "/opt/trn_rl_repo" not in sys.path:
    sys.path.insert(0, "/opt/trn_rl_repo")

import numpy as np

S, IN_DIM = 16384, 1024
NUM_PROJ, PROJ_DIM, BINS = 100, 64, 20
D = NUM_PROJ * PROJ_DIM          # 6400
N_CORES = 8
S_SHARD = S // N_CORES           # 2048
NE = BINS - 1                    # 19 interior edges (b = 1..19)
NP2 = 7                          # dual-edge passes on VectorE
NV = 2 * NP2                     # edges 1..14 on VectorE (pairs (b, b+7))
NA = NE - NV                     # edges 15..19 on ScalarE
PK = 4096.0                      # dual-edge packing factor

_CACHE = {}


def _cdf2_ref(in0, in1, s0, s1, imm2):
    x = in0.astype(np.float32)
    g = (x >= s0).astype(np.float32) + (x >= s1).astype(np.float32) * imm2
    return g, g.reshape(g.shape[0], -1).sum(axis=-1, keepdims=True)


def _register_cdf2():
    """Register the dual-edge compare+accumulate custom DVE op (idempotent)."""
    from operator import add

    from concourse import dve_ops
    from concourse.dve_spec import C0, C1, C2, Spec, Src0, Zero, lower
    from concourse.dve_uop import DveOpSpec

    if "CDF2_ANT" in dve_ops._SUB_OPCODE_FOR_NAME:
        return next(op for op in dve_ops.OPS if op.name == "CDF2_ANT")
    spec = Spec(
        body=(Src0 >= C0) + (Src0 >= C1) * C2,
        accum=add,
        accum_init=Zero,
        reference=_cdf2_ref,
    )
    row = max(dve_ops._SUB_OPCODE_FOR_NAME.values()) + 1
    assert row < 0x20
    sha = {}
    for ver in ("v3", "v4"):
        tmp = DveOpSpec(name="CDF2_ANT", opcode=row,
                        uops=lower(spec, ver=ver), rd1_en=False)
        sha[ver] = tmp.sha(ver)
    op = dve_ops.DveOp("CDF2_ANT", spec, subdim=False, uops_sha=sha)
    dve_ops.OPS.append(op)
    dve_ops.CUSTOM_DVE_SPECS[op.name] = op.spec
    dve_ops._SUB_OPCODE_FOR_NAME[op.name] = row
    return op


def build(s_shard=S_SHARD, d=D, in_dim=IN_DIM, n_cores=N_CORES, debug=False):
    import concourse.bacc as bacc
    import concourse.bass as bass
    from concourse import mybir
    from concourse.tile import TileContext

    cdf2 = _register_cdf2()

    f32 = mybir.dt.float32
    f16 = mybir.dt.float16
    i32 = mybir.dt.int32
    nt = d // 128                # 50
    kc_n = in_dim // 128         # 8
    chw = min(512, s_shard)      # matmul moving-operand width
    nch_n = s_shard // chw       # 4

    nc = bacc.Bacc("TRN2", target_bir_lowering=False, debug=False,
                   num_devices=n_cores)

    xT_d = nc.dram_tensor("xT16", [kc_n, 128, s_shard], f16,
                          kind="ExternalInput")
    wT_d = nc.dram_tensor("wT16", [kc_n, 128, d], f16, kind="ExternalInput")
    bias_d = nc.dram_tensor("bias", [128, nt], f32, kind="ExternalInput")
    out_d = nc.dram_tensor("out", [d, BINS], f32, kind="ExternalOutput")
    # pieces of the histogram all-reduce independently, overlapping compute;
    # the last piece is small so the trailing collective is short
    pieces = [(0, 22), (22, 42), (42, 47), (47, 49), (49, nt)]
    cc_ins, cc_outs = [], []
    for i, (t0, t1) in enumerate(pieces):
        cc_ins.append(nc.dram_tensor(f"cc_in{i}", [128, (t1 - t0) * BINS], f32))
        cc_outs.append(nc.dram_tensor(f"cc_out{i}", [128, (t1 - t0) * BINS],
                                      f32, addr_space="Shared"))
    if debug:
        dbg_hist = nc.dram_tensor("dbg_hist", [128, nt, BINS], f32,
                                  kind="ExternalOutput")
        dbg_cdf = nc.dram_tensor("dbg_cdf", [128, nt, BINS + 1], f32,
                                 kind="ExternalOutput")

    with TileContext(nc) as tc:
        with (
            tc.tile_pool(name="singles", bufs=1) as singles,
            tc.tile_pool(name="sp_pool", bufs=3) as sp_pool,
            tc.tile_pool(name="ps_p", bufs=2, space="PSUM") as ps_p,
        ):
            bias_sb = singles.tile([128, nt], f32)
            nc.sync.dma_start(out=bias_sb, in_=bias_d[:, :])

            # ScalarE Sign biases (-b for b = NV+1..19), one column each
            negb = singles.tile([128, NA], f32)
            for j in range(NA):
                nc.vector.memset(negb[:, j:j + 1], float(-(NV + 1 + j)))

            trash_v = singles.tile([128, s_shard], f32)
            trash_a = singles.tile([128, s_shard], f16)

            # per-engine cdf accumulators (separate tiles so Tile never
            # serializes VectorE against ScalarE on writes)
            accp = singles.tile([128, nt, NP2], f32)   # packed dual counts
            acc_a = singles.tile([128, nt, NA], f32)

            # ---- Phase 0: plain DMA of pre-transposed x shard and W ----
            xT = singles.tile([128, kc_n, s_shard], f16)
            wT = singles.tile([128, kc_n, d], f16)
            for nch in range(nch_n):
                sl = slice(nch * chw, (nch + 1) * chw)
                for kc in range(kc_n):
                    nc.sync.dma_start(out=xT[:, kc, sl], in_=xT_d[kc, :, sl])
            # W streams lazily in per-tile chunks behind the x load: tile 0
            # is gated only on x (4.2 MB) + the first 128 W columns, and
            # later chunks always arrive well ahead of their tile.
            for d0 in range(0, d, 128):
                for kc in range(kc_n):
                    nc.sync.dma_start(out=wT[:, kc, d0:d0 + 128],
                                      in_=wT_d[kc, :, d0:d0 + 128])

            # normalization / unpack scratch
            cdfx = singles.tile([128, nt, BINS + 1], f32)
            nc.vector.memset(cdfx[:, :, 0:1], float(s_shard))
            nc.vector.memset(cdfx[:, :, BINS:BINS + 1], 0.0)
            ip_t = singles.tile([128, nt, NP2], i32)
            ip2_t = singles.tile([128, nt, NP2], i32)
            hi_t = singles.tile([128, nt, NP2], f32)
            hist = singles.tile([128, nt, BINS], f32)
            hsum = singles.tile([128, nt, BINS], f32)
            sq = singles.tile([128, nt, BINS], f32)
            n2 = singles.tile([128, nt], f32)
            y_t = singles.tile([128, nt], f32)
            iy = singles.tile([128, nt], f32)
            a_t = singles.tile([128, nt], f32)
            b_t = singles.tile([128, nt], f32)
            r_t = singles.tile([128, nt], f32)
            outn = singles.tile([128, nt, BINS], f32)
            out_v = out_d[:, :].rearrange("(t p) b -> p t b", p=128)

            def emit_cc(hi):
                """Unpack cdf partials for tau in [t0, t1) and kick off the
                cross-core all-reduce (runs on DMA/CC queues in background)."""
                t0, t1 = pieces[hi]
                sl = slice(t0, t1)
                # unpack: lo = accp & 0xFFF (exact: accp is integer-valued
                # fp32 < 2^23, i32 convert exact), hi = (accp - lo) / 4096
                nc.vector.tensor_copy(ip_t[:, sl], accp[:, sl])
                nc.vector.tensor_scalar(ip2_t[:, sl], ip_t[:, sl], 0xFFF,
                                        None, op0=mybir.AluOpType.bitwise_and)
                nc.vector.tensor_copy(cdfx[:, sl, 1:1 + NP2], ip2_t[:, sl])
                nc.vector.tensor_tensor(out=hi_t[:, sl], in0=accp[:, sl],
                                        in1=cdfx[:, sl, 1:1 + NP2],
                                        op=mybir.AluOpType.subtract)
                nc.vector.tensor_scalar(cdfx[:, sl, 1 + NP2:1 + NV],
                                        hi_t[:, sl], 1.0 / PK, None,
                                        op0=mybir.AluOpType.mult)
                # ScalarE counts are sums of sign in {-1,0,1}:
                # cdf = 0.5*sgn + N/2
                nc.vector.tensor_scalar(
                    cdfx[:, sl, 1 + NV:BINS], acc_a[:, sl],
                    0.5, float(s_shard) / 2,
                    op0=mybir.AluOpType.mult, op1=mybir.AluOpType.add)
                nc.vector.tensor_tensor(
                    out=hist[:, sl], in0=cdfx[:, sl, 0:BINS],
                    in1=cdfx[:, sl, 1:BINS + 1],
                    op=mybir.AluOpType.subtract)
                if debug and hi == len(pieces) - 1:
                    nc.sync.dma_start(out=dbg_hist[:, :, :], in_=hist)
                    nc.sync.dma_start(out=dbg_cdf[:, :, :], in_=cdfx)
                nc.sync.dma_start(
                    out=cc_ins[hi][:, :],
                    in_=hist[:, sl].rearrange("p a b -> p (a b)"))
                nc.gpsimd.collective_compute(
                    "AllReduce",
                    mybir.AluOpType.add,
                    replica_groups=[list(range(n_cores))],
                    ins=[cc_ins[hi][:, :]],
                    outs=[cc_outs[hi][:, :]],
                )
                nc.sync.dma_start(
                    out=hsum[:, sl].rearrange("p a b -> p (a b)"),
                    in_=cc_outs[hi][:, :])

            hsum_g = singles.tile([128, nt, BINS], f32)

            def emit_norm(t0, t1, anchor=None):
                """L2-normalize the summed histogram for tau in [t0, t1) and
                write the output slice. When `anchor` is a tile index, route
                hsum through a no-op add of that tile's accumulator data so
                the scheduler's cost model places the chain after that tile's
                compares -- the collective is then long finished and no
                engine FIFO stalls on it."""
                sl = slice(t0, t1)
                w = t1 - t0
                if anchor is not None:
                    g_ap = acc_a[:, anchor, NA - 1:NA]
                    g_b = bass.AP(tensor=g_ap.tensor, offset=g_ap.offset,
                                  ap=[g_ap.ap[0], [0, w], [0, BINS]])
                    nc.vector.scalar_tensor_tensor(
                        out=hsum_g[:, sl], in0=g_b, scalar=0.0,
                        in1=hsum[:, sl],
                        op0=mybir.AluOpType.mult, op1=mybir.AluOpType.add)
                    h_in = hsum_g
                else:
                    h_in = hsum
                nc.vector.tensor_tensor(out=sq[:, sl], in0=h_in[:, sl],
                                        in1=h_in[:, sl],
                                        op=mybir.AluOpType.mult)
                nc.vector.tensor_reduce(out=n2[:, sl], in_=sq[:, sl],
                                        axis=mybir.AxisListType.X,
                                        op=mybir.AluOpType.add)
                nc.scalar.sqrt(y_t[:, sl], n2[:, sl])
                nc.vector.reciprocal(iy[:, sl], y_t[:, sl])
                # one Newton step for rsqrt: r = iy * (1.5 - 0.5*n2*iy^2)
                nc.vector.tensor_tensor(out=a_t[:, sl], in0=iy[:, sl],
                                        in1=iy[:, sl],
                                        op=mybir.AluOpType.mult)
                nc.vector.tensor_tensor(out=b_t[:, sl], in0=a_t[:, sl],
                                        in1=n2[:, sl],
                                        op=mybir.AluOpType.mult)
                nc.vector.tensor_scalar(b_t[:, sl], b_t[:, sl], -0.5, 1.5,
                                        op0=mybir.AluOpType.mult,
                                        op1=mybir.AluOpType.add)
                nc.vector.tensor_tensor(out=r_t[:, sl], in0=iy[:, sl],
                                        in1=b_t[:, sl],
                                        op=mybir.AluOpType.mult)
                r_ap = r_t[:, sl]
                r_b = bass.AP(tensor=r_ap.tensor, offset=r_ap.offset,
                              ap=[r_ap.ap[0], r_ap.ap[1], [0, BINS]])
                nc.vector.tensor_tensor(out=outn[:, sl], in0=h_in[:, sl],
                                        in1=r_b, op=mybir.AluOpType.mult)
                nc.sync.dma_start(out=out_v[:, sl], in_=outn[:, sl])

            # ---- Phase 1: d-tiles ----
            for tau in range(nt):
                pp = ps_p.tile([128, s_shard], f32)
                for nch in range(nch_n):
                    for kc in range(kc_n):
                        nc.tensor.matmul(
                            pp[:, nch * chw:(nch + 1) * chw],
                            lhsT=wT[:, kc, tau * 128:(tau + 1) * 128],
                            rhs=xT[:, kc, nch * chw:(nch + 1) * chw],
                            start=(kc == 0),
                            stop=(kc == kc_n - 1),
                        )
                # Stage PSUM -> SBUF fp16 once, adding the per-dim bias:
                # u = proj' - min * bins/width  in [0, bins].  Frees the
                # PSUM slot after ~2us so the PE is never gated on compares.
                sp = sp_pool.tile([128, s_shard], f16)
                nc.scalar.activation(
                    out=sp, in_=pp,
                    func=mybir.ActivationFunctionType.Identity,
                    bias=bias_sb[:, tau:tau + 1], scale=1.0)
                # VectorE: edges b and b+7 per dual pass (integer consts)
                for j in range(NP2):
                    nc.vector._custom_dve(
                        cdf2,
                        out=trash_v,
                        in0=sp,
                        s0=float(1 + j),
                        s1=float(1 + j + NP2),
                        imm2=PK,
                        accum_out=accp[:, tau, j:j + 1],
                    )
                # ScalarE: edges b = NV+1 .. 19 via Sign(u - b)
                for j in range(NA):
                    nc.scalar.activation(
                        out=trash_a,
                        in_=sp,
                        func=mybir.ActivationFunctionType.Sign,
                        bias=negb[:, j:j + 1],
                        scale=1.0,
                        accum_out=acc_a[:, tau, j:j + 1],
                    )
                for hi, (t0, t1) in enumerate(pieces):
                    if tau == t1 - 1:
                        emit_cc(hi)
            # normalization staged so the bulk runs during the last tiles'
            # compute (its collectives have long completed); only the last
            # piece's collective is actually waited on at the end
            emit_norm(0, 42, anchor=46)
            emit_norm(42, pieces[-2][1], anchor=nt - 1)
            emit_norm(pieces[-1][0], pieces[-1][1])

    nc.compile()
    return nc


def host_prep(x, W, mins, maxs, s_shard=S_SHARD, n_cores=N_CORES):
    d = W.shape[0]
    nt = d // 128
    kc_n = IN_DIM // 128
    mins64 = np.asarray(mins, dtype=np.float64)
    maxs64 = np.asarray(maxs, dtype=np.float64)
    scale = float(BINS) / (maxs64 - mins64)                        # [d]
    # W rows scaled so the matmul yields proj' = proj * bins/width
    Ws = np.asarray(W, dtype=np.float64) * scale[:, None]
    w16T = np.ascontiguousarray(Ws.T.astype(np.float16))           # [in, d]
    w16T = w16T.reshape(kc_n, 128, d)
    x16T = np.ascontiguousarray(np.asarray(x, dtype=np.float16).T)  # [in, S]
    x16T = x16T.reshape(kc_n, 128, S)
    bias = (-mins64 * scale).astype(np.float32)                    # [d]
    bias_l = np.ascontiguousarray(bias.reshape(nt, 128).T)         # [128, nt]
    in_maps = []
    for i in range(n_cores):
        in_maps.append({
            "xT16": np.ascontiguousarray(
                x16T[:, :, i * s_shard:(i + 1) * s_shard]),
            "wT16": w16T,
            "bias": bias_l,
        })
    return in_maps


def run(x, W, mins, maxs, trace=False, **trace_kw):
    """Returns (output [100, 64, 20] f32, BassKernelResults)."""
    from concourse.bass_utils import run_bass_kernel_spmd

    if "nc" not in _CACHE:
        _CACHE["nc"] = build()
    nc = _CACHE["nc"]
    in_maps = host_prep(x, W, mins, maxs)
    res = run_bass_kernel_spmd(nc, in_maps, core_ids=list(range(N_CORES)),
                               trace=trace, **trace_kw)
    out = res.results[0]["out"].reshape(NUM_PROJ, PROJ_DIM, BINS)
    return np.asarray(out, dtype=np.float32), res


def kernel(x, W, mins, maxs, num_of_projection=NUM_PROJ, bins=BINS):
    assert int(num_of_projection) == NUM_PROJ and int(bins) == BINS
    out, _ = run(x, W, mins, maxs, trace=False)
    return out
